# revision 19
# baseline (speedup 1.0000x reference)
"""GAT layer (nn_GATLayer) on 8 TRN2 NeuronCores — Bass/Tile kernel.

Math: out[i,h,:] = sum_j alpha[i,j,h] * Wx[j,h,:],
  alpha = softmax_j( mask(adj) leaky_relu(s_i + d_j) ) with
  s_i = (x W a_src)[i,h], d_j = (x W a_dst)[j,h].

Key factorization: exp(leaky(s+d)) = P_i*Q_j if s+d>0 else p_i*q_j, where
P=exp(s), p=exp(0.2 s), Q=exp(d), q=exp(0.2 d).  So with the binary branch
matrix B = adj * [s_i + d_j > 0]:
  out_unnorm = P_i * (B @ QWx) + p_i * ((adj @ qWx) - (B @ qWx))
  Z          = P_i * (B @ Q)   + p_i * ((adj @ q)   - (B @ q))
B is computed with a single fused DVE select per tile (custom TENSOR_MASK),
and every j-contraction is a PE matmul with {0,1}/f32r operands.

Sharding: rows i are split across 8 cores (512 each); x/W/a replicated;
each core receives its transposed adjacency slice adj[i_slice,:].T.
"""
import numpy as np

N_NODES, IN_F, OUT_F, H = 4096, 128, 32, 4
NCORES = 8
ROWS = N_NODES // NCORES          # 512 i-rows per core
JT = N_NODES // 128               # 32 j-tiles
NEG_SLOPE = 0.2

_cache = {}
last_results = None               # BassKernelResults of most recent run


def _build():
    import concourse.bass as bass
    import concourse.mybir as mybir
    import concourse.tile as tile
    from concourse import bacc
    from concourse.dve_ops import TENSOR_MASK

    F32 = mybir.dt.float32
    F32R = mybir.dt.float32r
    Exp = mybir.ActivationFunctionType.Exp
    Copy = mybir.ActivationFunctionType.Copy

    nc = bacc.Bacc("TRN2", target_bir_lowering=False)

    xT_h = nc.dram_tensor("xT", [IN_F, N_NODES], F32, kind="ExternalInput")
    xmy_h = nc.dram_tensor("xmyT", [IN_F, ROWS], F32, kind="ExternalInput")
    W_h = nc.dram_tensor("W", [IN_F, H * OUT_F], F32, kind="ExternalInput")
    WA8_h = nc.dram_tensor("WA8", [IN_F, 8], F32, kind="ExternalInput")
    nWAs_h = nc.dram_tensor("negWAs", [IN_F, 4], F32, kind="ExternalInput")
    adjm_h = nc.dram_tensor("adjm", [N_NODES, ROWS], F32R, kind="ExternalInput")
    id_h = nc.dram_tensor("ident", [128, 128], F32, kind="ExternalInput")
    out_h = nc.dram_tensor("out", [ROWS, H * OUT_F], F32, kind="ExternalOutput")

    with tile.TileContext(nc) as tc:
        import contextlib
        with contextlib.ExitStack() as ctx:
            const = ctx.enter_context(tc.tile_pool(name="const", bufs=1))
            big = ctx.enter_context(tc.tile_pool(name="big", bufs=1))
            mpool = ctx.enter_context(tc.tile_pool(name="mpool", bufs=6))
            bpool = ctx.enter_context(tc.tile_pool(name="bpool", bufs=12))
            cpool = ctx.enter_context(tc.tile_pool(name="cpool", bufs=3))
            psa = ctx.enter_context(tc.tile_pool(name="psa", bufs=2, space="PSUM"))
            psch_ctx = contextlib.ExitStack()
            psch = psch_ctx.enter_context(
                tc.tile_pool(name="psch", bufs=1, space="PSUM"))

            # ---- constants / inputs in SBUF ----
            xT = const.tile([IN_F, N_NODES], F32)
            for c in range(8):  # parallel DMA queues
                nc.sync.dma_start(xT[:, c * 512:(c + 1) * 512],
                                  xT_h[:, c * 512:(c + 1) * 512])
            xmy = const.tile([IN_F, ROWS], F32)
            nc.sync.dma_start(xmy[:], xmy_h[:, :])
            Wsb = const.tile([IN_F, H * OUT_F], F32)
            nc.sync.dma_start(Wsb[:], W_h[:, :])
            WA8 = const.tile([IN_F, 8], F32)
            nc.sync.dma_start(WA8[:], WA8_h[:, :])
            nWAs = const.tile([IN_F, 4], F32)
            nc.sync.dma_start(nWAs[:], nWAs_h[:, :])
            ident = const.tile([128, 128], F32)
            nc.sync.dma_start(ident[:], id_h[:, :])

            # ---- persistent big tensors ----
            # WxE: per j-tile, per head: [Wx_h (32) | ones (1)]
            WxE = big.tile([128, JT, H, 33], F32)
            nc.vector.memset(WxE[:, :, :, 32:33], 1.0)
            # scores in token layout: cols 0-3 = s (src), 4-7 = d (dst)
            scor = big.tile([128, JT, 8], F32)
            # Qq: cols 0-3 = Q_h = exp(d), 4-7 = q_h = exp(0.2 d)
            Qq = big.tile([128, JT, 8], F32)
            # ABw weights per (jt, h): [QWx(32) | Q | qWx(32) | q]
            ABw = big.tile([128, JT, H, 2, 33], F32R)
            # Mw: contiguous q-branch weight copies for the mask chains,
            # per pair pr: [qWx_{2pr} | q_{2pr} | qWx_{2pr+1} | q_{2pr+1}]
            Mw = big.tile([128, JT, 2, 66], F32R)
            # neg-src broadcast per head: [128, 512] (value -s_i on all parts)
            nsb = big.tile([128, H, ROWS], F32)
            # P/p per i-tile: cols 0-3 P_h = exp(s), 4-7 p_h
            Pp = big.tile([128, 4, 8], F32)

            # ---- negS rows + broadcast; P/p ----
            for h in range(H):
                pn = psa.tile([1, ROWS], F32, tag="psa")
                nc.tensor.matmul(pn[:], nWAs[:, h:h + 1], xmy[:],
                                 start=True, stop=True)
                nrow = const.tile([1, H, ROWS], F32, tag="nrow")
                nc.vector.tensor_copy(nrow[:, h, :], pn[:])
                nc.gpsimd.partition_broadcast(nsb[:, h, :], nrow[:, h, :])
            for it in range(4):
                pss = psa.tile([128, 8], F32, tag="psa")
                nc.tensor.matmul(pss[:], xmy[:, it * 128:(it + 1) * 128], WA8[:],
                                 start=True, stop=True)
                nc.scalar.activation(Pp[:, it, 0:4], pss[:, 0:4], Exp, scale=1.0)
                nc.scalar.activation(Pp[:, it, 4:8], pss[:, 0:4], Exp,
                                     scale=NEG_SLOPE)

            # ---- chain accumulators (PSUM, persistent) ----
            chAB = [psch.tile([66, ROWS], F32, tag=f"chAB{h}", name=f"chAB{h}") for h in range(H)]
            chM = [psch.tile([66, ROWS], F32, tag=f"chM{p}", name=f"chM{p}") for p in range(2)]

            # ---- main loop over j-tiles, in chunks of CH ----
            CH = 8
            for c0 in range(0, JT, CH):
                for jt in range(c0, c0 + CH):
                    # Wx + scores
                    ps = psa.tile([128, 136], F32, tag="psa")
                    nc.tensor.matmul(ps[:, 0:128],
                                     xT[:, jt * 128:(jt + 1) * 128],
                                     Wsb[:], start=True, stop=True)
                    nc.tensor.matmul(ps[:, 128:136],
                                     xT[:, jt * 128:(jt + 1) * 128],
                                     WA8[:], start=True, stop=True)
                    nc.scalar.copy(
                        WxE[:, jt, :, 0:32],
                        ps[:, 0:128].rearrange("p (h f) -> p h f", h=H))
                    nc.scalar.copy(scor[:, jt, :], ps[:, 128:136])
                # batched exp over the chunk's d-scores
                g = slice(c0, c0 + CH)
                nc.scalar.activation(Qq[:, g, 0:4], scor[:, g, 4:8], Exp,
                                     scale=1.0)
                nc.scalar.activation(Qq[:, g, 4:8], scor[:, g, 4:8], Exp,
                                     scale=NEG_SLOPE)

                for jt in range(c0, c0 + CH):
                    # mask tile [128 j, 512 i]
                    msb = mpool.tile([128, ROWS], F32R, tag="msb")
                    nc.sync.dma_start(msb[:], adjm_h[jt * 128:(jt + 1) * 128, :])
                    # weight build (one fused op): ABw[:, jt, h, br, :] =
                    #   [WxE_h | 1] * {Q_h (br=0), q_h (br=1)}
                    in0 = WxE[:, jt, :, :].unsqueeze(2).broadcast_to(
                        (128, H, 2, 33))
                    in1 = Qq[:, jt, :].rearrange("p (b h) -> p h b", b=2) \
                        .unsqueeze(3).broadcast_to((128, H, 2, 33))
                    nc.vector.tensor_mul(ABw[:, jt, :, :, :], in0, in1)
                    # contiguous q-branch weight copies (off the DVE: use DMA)
                    nc.sync.dma_start(
                        Mw[:, jt, :, :].rearrange("p a (b f) -> p (a b) f", b=2),
                        ABw[:, jt, :, 1, :])

                    # branch matrices + chain matmuls
                    st = (jt == 0)
                    sp = (jt == JT - 1)
                    for h in range(H):
                        B = bpool.tile([128, ROWS], F32R, tag="B")
                        nc.vector._custom_dve(
                            TENSOR_MASK, out=B[:], in0=msb[:].bitcast(F32),
                            in1=nsb[:, h, :], s0=scor[:, jt, 4 + h:5 + h],
                            imm2=0.0)
                        nc.tensor.matmul(chAB[h][:], ABw[:, jt, h, :, :], B[:],
                                         start=st, stop=sp)
                    for p in range(2):
                        nc.tensor.matmul(chM[p][:], Mw[:, jt, p, :], msb[:],
                                         start=st, stop=sp)

            # ---- epilogue: evac chains, transpose, combine ----
            chABs = [cpool.tile([66, ROWS], F32, tag=f"eAB{h}", name=f"eAB{h}") for h in range(H)]
            chMs = [cpool.tile([66, ROWS], F32, tag=f"eM{p}", name=f"eM{p}") for p in range(2)]
            for h in range(H):
                nc.scalar.copy(chABs[h][:], chAB[h][:])
            for p in range(2):
                nc.scalar.copy(chMs[p][:], chM[p][:])
            psch_ctx.close()  # release the 7 chain banks
            psc = ctx.enter_context(
                tc.tile_pool(name="psc", bufs=2, space="PSUM"))

            for it in range(4):
                sl = slice(it * 128, (it + 1) * 128)
                osb = cpool.tile([128, H * OUT_F], F32, tag="osb")
                tMs = []
                for pr in range(2):
                    tM = psc.tile([128, 66], F32, tag="tM", name=f"tM{pr}")
                    nc.tensor.transpose(tM[:], chMs[pr][:, sl],
                                        ident[0:66, 0:66])
                    tMs.append(tM)
                for h in range(H):
                    pr, hh = divmod(h, 2)
                    tM = tMs[pr]
                    tAB = psc.tile([128, 66], F32, tag="tAB")
                    nc.tensor.transpose(tAB[:], chABs[h][:, sl],
                                        ident[0:66, 0:66])
                    tABs = cpool.tile([128, 66], F32, tag="tABs")
                    nc.scalar.copy(tABs[:], tAB[:])
                    P_col = Pp[:, it, h:h + 1]
                    p_col = Pp[:, it, 4 + h:5 + h]
                    # u = P * [QWx-sums | Zpos]  (33 wide: cols 0:32 ∪ 64)
                    u = cpool.tile([128, 33], F32, tag="u")
                    nc.scalar.activation(u[:], tABs[:, 0:33], Copy,
                                         scale=P_col)
                    # v = (m-sums) - (B-sums) for the q branch
                    v = cpool.tile([128, 33], F32, tag="v")
                    nc.vector.tensor_sub(v[:, 0:32], tM[:, hh * 33:hh * 33 + 32],
                                         tABs[:, 33:65])
                    nc.vector.tensor_sub(v[:, 32:33],
                                         tM[:, hh * 33 + 32:hh * 33 + 33],
                                         tABs[:, 65:66])
                    # unn = u + p * v
                    w = cpool.tile([128, 33], F32, tag="w")
                    nc.vector.tensor_scalar_mul(w[:], v[:], p_col)
                    unn = cpool.tile([128, 33], F32, tag="unn")
                    nc.vector.tensor_add(unn[:], u[:], w[:])
                    rz = cpool.tile([128, 1], F32, tag="rz")
                    nc.vector.reciprocal(rz[:], unn[:, 32:33])
                    nc.vector.tensor_scalar_mul(
                        osb[:, h * OUT_F:(h + 1) * OUT_F], unn[:, 0:32], rz[:])
                nc.sync.dma_start(out_h[sl, :], osb[:])

    nc.compile()
    return nc


def _marshal(x, adj, W, a):
    x = np.asarray(x, dtype=np.float32)
    adj = np.asarray(adj)
    W = np.asarray(W, dtype=np.float32)
    a = np.asarray(a, dtype=np.float32)

    xT = np.ascontiguousarray(x.T)                       # [128, 4096]
    Wr = W.reshape(IN_F, H, OUT_F)
    WA8 = np.empty((IN_F, 8), dtype=np.float32)
    for h in range(H):
        WA8[:, h] = Wr[:, h, :] @ a[h, :OUT_F]           # src fold -> s
        WA8[:, 4 + h] = Wr[:, h, :] @ a[h, OUT_F:]       # dst fold -> d
    negWAs = np.ascontiguousarray(-WA8[:, 0:4])
    ident = np.eye(128, dtype=np.float32)
    adjT = adj.T.astype(np.float32)                      # [4096 j, 4096 i]

    in_maps = []
    for c in range(NCORES):
        sl = slice(c * ROWS, (c + 1) * ROWS)
        in_maps.append({
            "xT": xT,
            "xmyT": np.ascontiguousarray(xT[:, sl]),
            "W": W,
            "WA8": WA8,
            "negWAs": negWAs,
            "adjm": np.ascontiguousarray(adjT[:, sl]),
            "ident": ident,
        })
    return in_maps


def kernel(x, adj, W, a):
    global last_results
    from concourse.bass_utils import run_bass_kernel_spmd

    if "nc" not in _cache:
        _cache["nc"] = _build()
    nc = _cache["nc"]

    in_maps = _marshal(x, adj, W, a)
    res = run_bass_kernel_spmd(nc, in_maps, core_ids=list(range(NCORES)))
    last_results = res
    out = np.concatenate([r["out"] for r in res.results], axis=0)
    return out
